# revision 56
# baseline (speedup 1.0000x reference)
"""Ernie4 GQA attention layer as a Bass/Tile kernel for 8 TRN2 NeuronCores (v11).

Sharding: core c = 4*b + g handles batch b (of 2) and head-group g (of 4).
Each group owns 8 query heads + 1 kv head (GQA 32q/4kv, head_dim 128) and the
matching column slice of w_qkv / row slice of w_o. The o_proj partial sums are
reduced on the host (all-reduce equivalent).

Numerics (v4+): the two big GEMMs (qkv proj, o_proj) run as fp8e4 DoubleRow
matmuls (0.5 cycles/row, 2 contraction k-tiles per instruction = 4x bf16
FLOP rate) with 3-term error compensation:
    A@B ~= Aq@Bq + Ar@Bq + Aq@Br
where Aq = e4m3(A), Ar = e4m3(A - Aq) (unscaled residual), same for B.
Inputs are pre-scaled into e4m3's sweet spot (X*4, W*128, ctx*4) with the
scales folded into existing multiplicative knobs (psum->sbuf copy scale, the
0.25-scaled ones matrix for softmax denominators, the o_proj output copy
scale). Attention (scores, ctx) stays bf16. Output partials ship bf16.

Schedule (v11): attention q-block j=0 and ALL RoPE are interleaved into
phase 1's instruction stream. j=0 only needs key tiles 0..3 (ready after the
first 4-token-tile quarter) and is ACT/DVE-latency-bound standalone; inside
p1 its elementwise work rides under the fp8 matmul stream and its small PE
bits fill p1's DMA-gated gaps. RoPE runs in dve_add form (1 PE matmul per
chunk) per-quarter as soon as each 512-token chunk of rk/rq[h] is
transposed. Phase 2 is attention j=1..3 with o_proj of q-block j-1
interleaved (hb-major, 12 fp8-DR matmuls per item), plus the j=3 o_proj
tail. PSUM: p1(2) + transpose(2) + scores(2) + ctx(2) banks during p1;
scores2(2) + o_proj(2) replace p1+transpose in phase 2.
"""
import sys

sys.path.insert(0, "/opt/trn_rl_repo")

import numpy as np

HIDDEN = 4096
N_Q_HEADS = 32
N_KV_HEADS = 4
HEAD_DIM = 128
ROPE_THETA = 500000.0
Q_SIZE = N_Q_HEADS * HEAD_DIM  # 4096
KV_SIZE = N_KV_HEADS * HEAD_DIM  # 512
B = 2
S = 2048
N_CORES = 8
N_GROUPS = 4
HEADS_PER_GROUP = N_Q_HEADS // N_GROUPS  # 8
GROUP_Q = HEADS_PER_GROUP * HEAD_DIM  # 1024
QKV_G = GROUP_Q + 2 * HEAD_DIM  # 1280 columns of qkv per group
SCALE = HEAD_DIM ** -0.5
NK = HIDDEN // 128  # 32 contraction k-tiles
NKT = S // 128  # 16 token/key tiles per sequence
NQB = S // 512  # 4 q-blocks
NHB = HIDDEN // 512  # 8 output-hidden blocks

# fp8 compensation scales: X*4, W*128 -> qkv psum = 512*qkv;
# ctx*4 (via 0.25-scaled ones), wo*128 -> o_proj psum = 512*out.
X_PRE = 4.0
W_PRE = 128.0
QKV_SCL = 1.0 / (X_PRE * W_PRE)  # 2^-9 on qkv psum->sbuf copies
OST_SCL = 1.0 / (4.0 * W_PRE)  # 2^-9 on o_proj psum->out copies

_COMPILED = None
LAST_EXEC_NS = None
DEBUG_NO_INTERLEAVE = False
DEBUG_DUMP_CTX = False


def _build(phases=(1, 2, 3)):
    import concourse.mybir as mybir
    import concourse.tile as tile
    from concourse import bacc

    F32 = mybir.dt.float32
    F32R = mybir.dt.float32r
    BF16 = mybir.dt.bfloat16
    FP8 = mybir.dt.float8e4
    DR = mybir.MatmulPerfMode.DoubleRow

    nc = bacc.Bacc("TRN2", target_bir_lowering=False, debug=False, num_devices=N_CORES)

    # xq/xr: [tt, 128, NK, 128]; x*[tt, h, ko, t] = e4m3 hi/lo of
    # X_PRE * X[tt*128+t, ko*128+h] (contiguous 4KB per partition row)
    xq = nc.dram_tensor("xq", [NKT, 128, NK, 128], FP8, kind="ExternalInput").ap()
    xr = nc.dram_tensor("xr", [NKT, 128, NK, 128], FP8, kind="ExternalInput").ap()
    # wq/wr split per feature block (contiguous -> full-rate DMA descriptors):
    # w[s][fb]: [128, NK, fw]; hi/lo e4m3 of W_PRE*W^T[ko*128+p, c0+f]
    w_dram = {}
    for s in ("q", "r"):
        for fb, (c0, fw) in enumerate(((0, 256), (256, 512), (768, 512))):
            w_dram[(s, fb)] = nc.dram_tensor(
                f"w{s}{fb}", [128, NK, fw], FP8, kind="ExternalInput"
            ).ap()
    # woq/wor: [128, hk, 4096]; hi/lo e4m3 of W_PRE*w_o[o, g*1024+hk*128+p]
    woq = nc.dram_tensor("woq", [128, HEADS_PER_GROUP, HIDDEN], FP8,
                         kind="ExternalInput").ap()
    wor = nc.dram_tensor("wor", [128, HEADS_PER_GROUP, HIDDEN], FP8,
                         kind="ExternalInput").ap()
    # rope tables (bf16): cos2[2i]=cos2[2i+1]=cos; sin2[2i]=+sin, sin2[2i+1]=-sin
    cos2 = nc.dram_tensor("cos2", [HEAD_DIM, S], BF16, kind="ExternalInput").ap()
    sin2 = nc.dram_tensor("sin2", [HEAD_DIM, S], BF16, kind="ExternalInput").ap()
    swp = nc.dram_tensor("swp", [128, 128], BF16, kind="ExternalInput").ap()
    # ones scaled by 0.25 so rcp = 4/sum(p) and ctxb = 4*ctx (e4m3 sweet spot)
    ones = nc.dram_tensor("ones", [128, 128], F32R, kind="ExternalInput").ap()
    ident = nc.dram_tensor("ident", [128, 128], BF16, kind="ExternalInput").ap()
    # triangular edge mask: maskt[p, c] = 1 if c >= p else 0
    maskt = nc.dram_tensor("maskt", [128, 128], BF16, kind="ExternalInput").ap()
    out_part = nc.dram_tensor(
        "out_part", [NKT, NHB, 128, 512], BF16, kind="ExternalOutput"
    ).ap()
    ctx_dbg = None
    if DEBUG_DUMP_CTX:
        ctx_dbg = nc.dram_tensor(
            "ctx_dbg", [NQB, HEADS_PER_GROUP, 128, 512], BF16,
            kind="ExternalOutput"
        ).ap()

    FB = ((0, 256), (256, 512), (768, 512))  # (col0, width): kv | q0..3 | q4..7

    from contextlib import ExitStack

    with tile.TileContext(nc) as tc:
        with ExitStack() as _stk:
            # attention pools live for the whole kernel: q-block j=0 runs
            # interleaved inside phase 1
            cpool = _stk.enter_context(tc.tile_pool(name="consts", bufs=1))
            vpool = _stk.enter_context(tc.tile_pool(name="vsb", bufs=1))
            rqpool = _stk.enter_context(tc.tile_pool(name="rqsb", bufs=1))
            ropet = _stk.enter_context(tc.tile_pool(name="ropet", bufs=2))
            ptpool = _stk.enter_context(tc.tile_pool(name="pt", bufs=4))
            paccpool = _stk.enter_context(tc.tile_pool(name="pacc", bufs=2))
            rcppool = _stk.enter_context(tc.tile_pool(name="rcp", bufs=2))
            ctmppool = _stk.enter_context(tc.tile_pool(name="ctmp", bufs=2))
            ctxq8pool = _stk.enter_context(tc.tile_pool(name="ctxq8", bufs=2))
            ctxr8pool = _stk.enter_context(tc.tile_pool(name="ctxr8", bufs=2))
            scps = _stk.enter_context(tc.tile_pool(name="scps", bufs=2, space="PSUM"))
            ctxps = _stk.enter_context(tc.tile_pool(name="ctxps", bufs=2, space="PSUM"))
            swp_sb = cpool.tile([128, 128], BF16)
            ones_sb = cpool.tile([128, 128], F32R)
            id_sb = cpool.tile([128, 128], BF16)
            mask_sb = cpool.tile([128, 128], BF16)
            cos_sb = cpool.tile([128, S], BF16)
            sin_sb = cpool.tile([128, S], BF16)

            # persistent SBUF state
            v_sb = [vpool.tile([128, 128], BF16, name=f"v{i}") for i in range(NKT)]
            rq = [rqpool.tile([128, S], BF16, name=f"rq{i}") for i in range(HEADS_PER_GROUP)]
            rk = rqpool.tile([128, S], BF16)

            def rope_chunk(t, c, pspool, pstag):
                # t[:, cs] = swp@(t*sin') + t*cos; DVE does the final add
                cs = slice(c * 512, (c + 1) * 512)
                m_sin = ropet.tile([128, 512], BF16, tag="msin", name="msin")
                nc.vector.tensor_mul(m_sin, t[:, cs], sin_sb[:, cs])
                m_cos = ropet.tile([128, 512], BF16, tag="mcos", name="mcos")
                nc.vector.tensor_mul(m_cos, t[:, cs], cos_sb[:, cs])
                ps = pspool.tile([128, 512], F32, tag=pstag, name="ropeps")
                nc.tensor.matmul(ps, swp_sb, m_sin, start=True, stop=True)
                nc.vector.tensor_add(t[:, cs], ps, m_cos)

            def finalize(fin, pool_merge=False):
                pacc_a, pacc_b, ctx_ps, ctx_q, ctx_r, jh = fin
                acc = pacc_a
                if pacc_b is not None:
                    eng = nc.gpsimd if pool_merge else nc.vector
                    eng.tensor_add(
                        pacc_a, pacc_a.bitcast(F32), pacc_b.bitcast(F32)
                    )
                r_ps = scps.tile([128, 512], F32, tag="sc", name="rpst")
                nc.tensor.matmul(r_ps, ones_sb, acc, start=True, stop=True)
                rcp = rcppool.tile([128, 512], F32, tag="rcp", name="rcpt")
                nc.vector.reciprocal(rcp, r_ps)
                # ctxb = 4*ctx (ones are 0.25-scaled); fp8 split for o_proj
                ctmp = ctmppool.tile([128, 512], BF16, tag="ctmp", name="ctmpt")
                nc.vector.tensor_mul(ctmp, ctx_ps, rcp)
                nc.gpsimd.tensor_copy(ctx_q, ctmp)
                nc.vector.tensor_tensor(
                    ctx_r, ctmp, ctx_q, mybir.AluOpType.subtract
                )
                if ctx_dbg is not None:
                    nc.sync.dma_start(ctx_dbg[jh[0], jh[1]], ctmp)

            class AttnBlock:
                """Emits attention for one q-block j, one (head, key-tile)
                step at a time. step() returns False when exhausted."""

                def __init__(self, j, sc_pool2=None, op=None, op_step=0.0):
                    self.j = j
                    self.nkt = 4 * (j + 1)
                    self.kt_order = (list(range(4 * j, 4 * j + 4))
                                     + list(range(4 * j)))
                    self.ctx_q8 = ctxq8pool.tile(
                        [128, HEADS_PER_GROUP, 512], FP8, tag="ctxq", name="ctxq8"
                    )
                    self.ctx_r8 = ctxr8pool.tile(
                        [128, HEADS_PER_GROUP, 512], FP8, tag="ctxr", name="ctxr8"
                    )
                    self.sc_pool2 = sc_pool2
                    self.op = op
                    self.op_budget = 0.0
                    self.op_step = op_step
                    self.h = 0
                    self.i = 0
                    self.fin = None
                    self.pend = []
                    self.sc_flip = 0
                    self._start_head()

                def _sc_emit(self, i):
                    kt = self.kt_order[i]
                    di = kt - 4 * self.j
                    col0 = di * 128 if di >= 0 else 0
                    pool = scps if (self.sc_pool2 is None or self.sc_flip == 0) \
                        else self.sc_pool2
                    self.sc_flip ^= 1
                    sc_ps = pool.tile([128, 512], F32, tag="sc", name="scpst")
                    nc.tensor.matmul(
                        sc_ps[:, col0:],
                        rk[:, kt * 128:(kt + 1) * 128],
                        rq[self.h][:, self.j * 512 + col0:(self.j + 1) * 512],
                        start=True,
                        stop=True,
                    )
                    return sc_ps, kt, col0

                def _start_head(self):
                    self.ctx_ps = ctxps.tile([128, 512], F32, tag="ctxp", name="ctxpt")
                    self.pacc_a = paccpool.tile(
                        [128, 512], F32R, tag="pacca", name="pacca"
                    )
                    self.pacc_b = None
                    self.b_init = False
                    self.flip = False
                    self.pend = [self._sc_emit(0)]
                    if self.sc_pool2 is not None and self.nkt > 1:
                        self.pend.append(self._sc_emit(1))
                    if self.fin is not None:
                        finalize(self.fin, pool_merge=(self.j == 0))
                        self.fin = None

                def step(self):
                    if self.h >= HEADS_PER_GROUP:
                        return False
                    i, j, nkt = self.i, self.j, self.nkt
                    sc_ps, kt, col0 = self.pend.pop(0)
                    di = kt - 4 * j
                    pt = ptpool.tile([128, 512], BF16, tag="pt", name="ptt")
                    nc.scalar.activation(
                        pt[:, col0:], sc_ps[:, col0:],
                        mybir.ActivationFunctionType.Exp,
                        scale=SCALE,
                    )
                    if di >= 0:  # diagonal: mask triangular edge
                        nc.vector.tensor_mul(
                            pt[:, col0:col0 + 128],
                            pt[:, col0:col0 + 128],
                            mask_sb,
                        )
                    # dual-chain denominator accumulation
                    if i == 0:
                        nc.gpsimd.tensor_copy(self.pacc_a, pt)
                    elif col0 == 0 and not self.b_init:
                        self.pacc_b = paccpool.tile(
                            [128, 512], F32R, tag="paccb", name="paccb"
                        )
                        nc.vector.tensor_copy(self.pacc_b, pt)
                        self.b_init = True
                    elif (not self.b_init) or self.flip:
                        nc.gpsimd.tensor_add(
                            self.pacc_a[:, col0:],
                            self.pacc_a[:, col0:].bitcast(F32),
                            pt[:, col0:],
                        )
                        self.flip = False
                    else:
                        nc.vector.tensor_add(
                            self.pacc_b[:, col0:],
                            self.pacc_b[:, col0:].bitcast(F32),
                            pt[:, col0:],
                        )
                        self.flip = True
                    if self.op is not None:
                        self.op_budget += self.op_step
                        n = int(self.op_budget)
                        self.op_budget -= n
                        self.op.emit(n)
                    nc.tensor.matmul(
                        self.ctx_ps[:, col0:],
                        v_sb[kt],
                        pt[:, col0:],
                        start=(i == 0),
                        stop=(i == nkt - 1),
                        skip_group_check=True,
                    )
                    depth = 2 if self.sc_pool2 is not None else 1
                    if i + depth < nkt:
                        self.pend.append(self._sc_emit(i + depth))
                    self.i += 1
                    if self.i == nkt:
                        self.fin = (self.pacc_a, self.pacc_b, self.ctx_ps,
                                    self.ctx_q8[:, self.h, :],
                                    self.ctx_r8[:, self.h, :],
                                    (self.j, self.h))
                        self.h += 1
                        self.i = 0
                        if self.h < HEADS_PER_GROUP:
                            self._start_head()
                    return True

                def finish(self):
                    while self.step():
                        pass

                def flush_fin(self):
                    if self.fin is not None:
                        finalize(self.fin, pool_merge=(self.j == 0))
                        self.fin = None

            # ---------------- phase 1: qkv + j0 attention + all rope --------
            j0 = None
            if 2 in phases:
                pass  # j0 created mid-p1 once quarter 0's data is ready
            with ExitStack() as _stk1:
                wpool = _stk1.enter_context(tc.tile_pool(name="w", bufs=1))
                xqpool = _stk1.enter_context(tc.tile_pool(name="xq", bufs=4))
                xrpool = _stk1.enter_context(tc.tile_pool(name="xr", bufs=4))
                qspool = _stk1.enter_context(tc.tile_pool(name="qs", bufs=3))
                ktpool = _stk1.enter_context(tc.tile_pool(name="kt8", bufs=2))
                p1ps = _stk1.enter_context(tc.tile_pool(name="p1ps", bufs=2, space="PSUM"))
                tpps = _stk1.enter_context(tc.tile_pool(name="tp", bufs=2, space="PSUM"))
                w_tiles = {}
                x_tiles = {}

                def emit_x(tt):
                    # X rides the second HWDGE queue (ACT) so it fair-shares
                    # the DMA engines with the W stream on SP
                    xq_t = xqpool.tile([128, NK, 128], FP8, tag="xq", name="xqt")
                    nc.scalar.dma_start(xq_t, xq[tt])
                    xr_t = xrpool.tile([128, NK, 128], FP8, tag="xr", name="xrt")
                    nc.scalar.dma_start(xr_t, xr[tt])
                    x_tiles[tt] = (xq_t, xr_t)

                def p1_preamble():
                    for s in ("q", "r"):
                        for fb, (c0, fw) in enumerate(FB):
                            for kc in range(4):
                                wt = wpool.tile([128, 8, fw], FP8, name=f"w{s}{fb}_{kc}")
                                w_tiles[(s, fb, kc)] = wt

                    def wdma(s, fb, kc):
                        nc.sync.dma_start(
                            w_tiles[(s, fb, kc)],
                            w_dram[(s, fb)][:, kc * 8:(kc + 1) * 8, :],
                        )

                    # first W chunk in two halves so the PE starts sooner
                    nc.sync.dma_start(
                        w_tiles[("q", 0, 0)][:, :4, :], w_dram[("q", 0)][:, 0:4, :]
                    )
                    # x0 in two halves so the first kv matmuls start sooner
                    xq_t0 = xqpool.tile([128, NK, 128], FP8, tag="xq", name="xqt0")
                    nc.scalar.dma_start(xq_t0[:, :8], xq[0][:, :8])
                    nc.sync.dma_start(
                        w_tiles[("q", 0, 0)][:, 4:, :], w_dram[("q", 0)][:, 4:8, :]
                    )
                    nc.scalar.dma_start(xq_t0[:, 8:16], xq[0][:, 8:16])
                    wdma("q", 0, 1)
                    nc.scalar.dma_start(xq_t0[:, 16:], xq[0][:, 16:])
                    wdma("q", 0, 2), wdma("q", 0, 3)
                    xr_t0 = xrpool.tile([128, NK, 128], FP8, tag="xr", name="xrt0")
                    nc.scalar.dma_start(xr_t0, xr[0])
                    x_tiles[0] = (xq_t0, xr_t0)
                    for kc in range(4):
                        wdma("r", 0, kc)
                    nc.sync.dma_start(id_sb, ident)
                    emit_x(1)
                    emit_x(2)
                    emit_x(3)
                    for kc in range(4):
                        wdma("q", 1, kc)
                    for kc in range(4):
                        wdma("r", 1, kc)
                    for kc in range(4):
                        wdma("q", 2, kc)
                    for kc in range(4):
                        wdma("r", 2, kc)
                    # consts needed by rope/attention interleave from quarter 1
                    nc.sync.dma_start(swp_sb, swp)
                    nc.sync.dma_start(ones_sb, ones)
                    nc.sync.dma_start(mask_sb, maskt)
                    nc.sync.dma_start(cos_sb, cos2)
                    nc.sync.dma_start(sin_sb, sin2)

                def transpose_to(src):
                    tps = tpps.tile([128, 128], BF16, tag="tp", name="tps")
                    nc.tensor.transpose(tps, src, id_sb)
                    return tps

                pend_q = []  # deferred q transposes: (qs_tile, hh_base, tt)

                def p1_block(tt, fb):
                    c0, fw = FB[fb]
                    xq_t, xr_t = x_tiles[tt]
                    ps = p1ps.tile([128, 512], F32, tag="p1", name="p1t")
                    n_mm = 3 * (NK // 2)
                    mi = 0
                    # term order AqBq, ArBq, AqBr: xr arrives before wr via DMA
                    for a_t, w_s in ((xq_t, "q"), (xr_t, "q"), (xq_t, "r")):
                        for i in range(NK // 2):
                            kc, m = divmod(i, 4)
                            nc.tensor.matmul(
                                ps[:, :fw],
                                a_t[:, 2 * i:2 * i + 2, :],
                                w_tiles[(w_s, fb, kc)][:, 2 * m:2 * m + 2, :],
                                start=(mi == 0),
                                stop=(mi == n_mm - 1),
                                perf_mode=DR,
                            )
                            mi += 1
                    if fb == 0:
                        kt8 = ktpool.tile([128, 128], BF16, tag="kt8", name="kt8t")
                        nc.vector.tensor_scalar_mul(kt8, ps[:, :128], QKV_SCL)
                        nc.vector.tensor_scalar_mul(v_sb[tt], ps[:, 128:256], QKV_SCL)
                        tps = transpose_to(kt8)
                        nc.scalar.copy(rk[:, tt * 128:(tt + 1) * 128], tps)
                    else:
                        qs = qspool.tile([128, 512], BF16, tag="qs", name="qst")
                        nc.vector.tensor_scalar_mul(qs, ps[:, :fw], QKV_SCL)
                        pend_q.append((qs, (fb - 1) * 4, tt))
                    # drain one pending q-transpose batch per block
                    if len(pend_q) > 1:
                        qs_t, hh0, qtt = pend_q.pop(0)
                        for hh in range(4):
                            h = hh0 + hh
                            tps = transpose_to(qs_t[:, hh * 128:(hh + 1) * 128])
                            nc.scalar.copy(
                                rq[h][:, qtt * 128:(qtt + 1) * 128], tps
                            )

                def p1_run():
                    # rope chunk c of a tensor needs the transposes of token
                    # tiles 4c..4c+3: heads 0-3 drain by end of quarter c,
                    # heads 4-7 two blocks into quarter c+1
                    nonlocal j0
                    rope_ok = 2 in phases
                    groups = [(0, 4), (4, 8), (8, 12), (12, 16)]
                    rope_lo = 0  # chunks roped for rk/rq0-3
                    rope_hi = 0  # chunks roped for rq4-7
                    for gi, (g0, g1) in enumerate(groups):
                        nblk = 0
                        for fb in range(3):
                            for tt in range(g0, g1):
                                p1_block(tt, fb)
                                nblk += 1
                                # prefetch at fb2: with bufs=4 the new tile
                                # reuses x(tt)'s buffer, whose last reader
                                # (this very block) is now emitted -- the pool
                                # WAR tracking only sees already-emitted reads
                                if fb == 2 and tt + 4 < NKT:
                                    emit_x(tt + 4)
                                if gi > 0 and nblk == 2 and rope_ok:
                                    # force-drain pending transpose batches:
                                    # the len>1 guard leaves the last (h4-7,
                                    # tt_{4c+3}) batch pending, and roping a
                                    # chunk before its transposes are emitted
                                    # bakes un-roped q into those columns
                                    while pend_q:
                                        qs_t, hh0, qtt = pend_q.pop(0)
                                        for hh in range(4):
                                            h = hh0 + hh
                                            tps = transpose_to(
                                                qs_t[:, hh * 128:(hh + 1) * 128])
                                            nc.scalar.copy(
                                                rq[h][:, qtt * 128:(qtt + 1) * 128],
                                                tps)
                                    while rope_hi < rope_lo:
                                        for h in range(4, 8):
                                            rope_chunk(rq[h], rope_hi, p1ps, "p1")
                                        rope_hi += 1
                                    if j0 is None:
                                        j0 = AttnBlock(0)
                                # paced j0 attention: its ACT/DVE chains ride
                                # under the fp8 matmul stream
                                if j0 is not None and nblk >= 3 and not DEBUG_NO_INTERLEAVE:
                                    j0.step()
                        if rope_ok:
                            rope_chunk(rk, gi, p1ps, "p1")
                            for h in range(4):
                                rope_chunk(rq[h], gi, p1ps, "p1")
                            rope_lo = gi + 1
                    return rope_hi, rope_lo

                rope_hi = rope_lo = 0
                if 1 in phases:
                    p1_preamble()
                    rope_hi, rope_lo = p1_run()
                for qs_t, hh0, qtt in pend_q:
                    for hh in range(4):
                        h = hh0 + hh
                        tps = transpose_to(qs_t[:, hh * 128:(hh + 1) * 128])
                        nc.scalar.copy(rq[h][:, qtt * 128:(qtt + 1) * 128], tps)
                if 1 in phases and 2 in phases:
                    # rope leftovers: heads 4-7 chunk 3
                    while rope_hi < rope_lo:
                        for h in range(4, 8):
                            rope_chunk(rq[h], rope_hi, p1ps, "p1")
                        rope_hi += 1
                    if j0 is None:
                        j0 = AttnBlock(0)
                    j0.finish()
                    j0.flush_fin()

            # ---------------- phase 2/3: attention j>=1 + o_proj ------------
            with ExitStack() as _stk2:
                wopool = _stk2.enter_context(tc.tile_pool(name="wo", bufs=1))
                ostpool = _stk2.enter_context(tc.tile_pool(name="ost", bufs=4))
                scps2 = _stk2.enter_context(tc.tile_pool(name="sc2", bufs=2, space="PSUM"))
                opps = _stk2.enter_context(tc.tile_pool(name="opps", bufs=2, space="PSUM"))
                woq_sb = wopool.tile([128, HEADS_PER_GROUP, HIDDEN], FP8)
                wor_sb = wopool.tile([128, HEADS_PER_GROUP, HIDDEN], FP8)
                if 3 in phases:
                    # hb-sliced in o_proj emission order
                    for hb in range(NHB):
                        nc.sync.dma_start(
                            woq_sb[:, :, hb * 512:(hb + 1) * 512],
                            woq[:, :, hb * 512:(hb + 1) * 512],
                        )
                        nc.sync.dma_start(
                            wor_sb[:, :, hb * 512:(hb + 1) * 512],
                            wor[:, :, hb * 512:(hb + 1) * 512],
                        )

                class OpEmitter:
                    def __init__(self, j, ctx_q8, ctx_r8):
                        self.items = [
                            (tl, hb)
                            for hb in range(NHB)
                            for tl in range(4)
                        ] if (3 in phases) else []
                        self.j = j
                        self.cq = ctx_q8
                        self.cr = ctx_r8
                        self.pos = 0

                    def emit(self, n):
                        # n counts DR-matmul triples (one head-pair, 3 terms)
                        for _ in range(n):
                            if self.pos >= 4 * len(self.items):
                                return
                            item, hp = divmod(self.pos, 4)
                            tl, hb = self.items[item]
                            ts = slice(tl * 128, (tl + 1) * 128)
                            hs = slice(2 * hp, 2 * hp + 2)
                            os_ = slice(hb * 512, (hb + 1) * 512)
                            if hp == 0:
                                self.ps = opps.tile([128, 512], F32, tag="op", name="opps")
                            for a_t, w_t in (
                                (self.cq, woq_sb),
                                (self.cq, wor_sb),
                                (self.cr, woq_sb),
                            ):
                                nc.tensor.matmul(
                                    self.ps,
                                    a_t[:, hs, ts],
                                    w_t[:, hs, os_],
                                    start=(hp == 0 and a_t is self.cq and w_t is woq_sb),
                                    stop=(hp == 3 and a_t is self.cr),
                                    perf_mode=DR,
                                )
                            if hp == 3:
                                ost = ostpool.tile([128, 512], BF16, tag="ost", name="ost")
                                if item % 2 == 0:
                                    nc.vector.tensor_scalar_mul(ost, self.ps, OST_SCL)
                                else:
                                    nc.scalar.activation(
                                        ost, self.ps,
                                        mybir.ActivationFunctionType.Copy,
                                        scale=OST_SCL,
                                    )
                                nc.sync.dma_start(
                                    out_part[self.j * 4 + tl, hb], ost
                                )
                            self.pos += 1

                    def flush(self):
                        self.emit(4 * len(self.items) - self.pos)

                prev_op = None
                if 2 in phases and 3 in phases and j0 is not None:
                    prev_op = OpEmitter(0, j0.ctx_q8, j0.ctx_r8)
                for j in range(1, NQB) if 2 in phases else []:
                    nkt_j = 4 * (j + 1)
                    # 128 DR-triples per j, paced over 8*nkt_j attention steps;
                    # j=1 starts with a small deficit so the first wo slices
                    # can land after the w pool frees
                    op_step = (16.0 / nkt_j) if prev_op is not None else 0.0
                    if DEBUG_NO_INTERLEAVE and prev_op is not None:
                        prev_op.flush()
                    blk = AttnBlock(j, sc_pool2=scps2,
                                    op=None if DEBUG_NO_INTERLEAVE else prev_op,
                                    op_step=op_step)
                    if prev_op is not None and j == 1:
                        blk.op_budget = -6.0
                    blk.finish()
                    blk.flush_fin()
                    if prev_op is not None:
                        prev_op.flush()
                    prev_op = OpEmitter(j, blk.ctx_q8, blk.ctx_r8)
                if 2 in phases and prev_op is not None:
                    prev_op.flush()

    nc.compile()
    return nc


def _host_inputs(positions, hidden_states, w_qkv, w_o):
    """Shard + fp8-split + lay out inputs for the 8 cores (c = 4*b + g)."""
    import ml_dtypes

    bf16 = ml_dtypes.bfloat16
    fp8 = ml_dtypes.float8_e4m3
    positions = np.asarray(positions)
    hidden_states = np.asarray(hidden_states, dtype=np.float32)
    w_qkv = np.asarray(w_qkv, dtype=np.float32)
    w_o = np.asarray(w_o, dtype=np.float32)

    def split8(a):
        hi = a.astype(fp8)
        lo = (a - hi.astype(np.float32)).astype(fp8)
        return hi, lo

    inv_freq = 1.0 / (ROPE_THETA ** (np.arange(0, HEAD_DIM, 2, dtype=np.float64) / HEAD_DIM))
    ang = positions.astype(np.float64)[None, :] * inv_freq[:, None]  # [half, S]
    c = np.cos(ang).astype(np.float32)
    s = np.sin(ang).astype(np.float32)
    cos2 = np.empty((HEAD_DIM, S), dtype=np.float32)
    sin2 = np.empty((HEAD_DIM, S), dtype=np.float32)
    cos2[0::2] = c
    cos2[1::2] = c
    sin2[0::2] = s
    sin2[1::2] = -s

    swp = np.zeros((128, 128), dtype=np.float32)
    idx = np.arange(0, 128, 2)
    swp[idx, idx + 1] = 1.0
    swp[idx + 1, idx] = 1.0
    ones = np.full((128, 128), 0.25, dtype=np.float32)
    ident = np.eye(128, dtype=np.float32)
    maskt = (np.arange(128)[None, :] >= np.arange(128)[:, None]).astype(np.float32)

    xqs, xrs = [], []
    for b in range(B):
        xt_t = np.ascontiguousarray(
            (X_PRE * hidden_states[b]).reshape(NKT, 128, NK, 128).transpose(0, 3, 2, 1)
        )  # [tt, h, ko, t] f32
        hi, lo = split8(xt_t)
        xqs.append(hi)
        xrs.append(lo)

    wqs, wrs, woqs, wors = [], [], [], []
    for g in range(N_GROUPS):
        cols = np.concatenate([
            np.arange(Q_SIZE + g * HEAD_DIM, Q_SIZE + (g + 1) * HEAD_DIM),  # k
            np.arange(Q_SIZE + KV_SIZE + g * HEAD_DIM, Q_SIZE + KV_SIZE + (g + 1) * HEAD_DIM),  # v
            np.arange(g * GROUP_Q, (g + 1) * GROUP_Q),  # q0..q7
        ])
        wq_g = W_PRE * w_qkv[cols, :]  # [1280, 4096]
        wqkvt_t = np.ascontiguousarray(
            wq_g.T.reshape(NK, 128, QKV_G).transpose(1, 0, 2)
        )
        hi, lo = split8(wqkvt_t)
        wqs.append(hi)
        wrs.append(lo)  # each [128, NK, 1280]; sliced per fb below
        wot_full = W_PRE * w_o[:, g * GROUP_Q:(g + 1) * GROUP_Q].T  # [1024, 4096]
        wot_t = np.ascontiguousarray(
            wot_full.reshape(HEADS_PER_GROUP, 128, HIDDEN).transpose(1, 0, 2)
        )
        hi, lo = split8(wot_t)
        woqs.append(hi)
        wors.append(lo)

    FBH = ((0, 256), (256, 512), (768, 512))
    in_maps = []
    for c_id in range(N_CORES):
        b, g = divmod(c_id, N_GROUPS)
        wmap = {}
        for s, arr in (("q", wqs[g]), ("r", wrs[g])):
            for fb, (c0, fw) in enumerate(FBH):
                wmap[f"w{s}{fb}"] = np.ascontiguousarray(arr[:, :, c0:c0 + fw])
        in_maps.append({
            "xq": xqs[b],
            "xr": xrs[b],
            **wmap,
            "woq": woqs[g],
            "wor": wors[g],
            "cos2": cos2.astype(bf16),
            "sin2": sin2.astype(bf16),
            "swp": swp.astype(bf16),
            "ones": ones,
            "ident": ident.astype(bf16),
            "maskt": maskt.astype(bf16),
        })
    return in_maps


def kernel(positions, hidden_states, w_qkv, w_o):
    global _COMPILED, LAST_EXEC_NS
    from concourse import bass_utils

    if _COMPILED is None:
        _COMPILED = _build()
    nc = _COMPILED

    in_maps = _host_inputs(positions, hidden_states, w_qkv, w_o)
    res = bass_utils.run_bass_kernel_spmd(
        nc, in_maps, core_ids=list(range(N_CORES))
    )
    LAST_EXEC_NS = res.exec_time_ns

    out = np.zeros((B, S, HIDDEN), dtype=np.float32)
    for c_id in range(N_CORES):
        b = c_id // N_GROUPS
        part = res.results[c_id]["out_part"]  # [NKT, NHB, 128, 512] bf16
        out[b] += part.astype(np.float32).transpose(0, 2, 1, 3).reshape(S, HIDDEN)
    return out


# revision 64
# speedup vs baseline: 1.0011x; 1.0011x over previous
"""Ernie4 GQA attention layer as a Bass/Tile kernel for 8 TRN2 NeuronCores (v11).

Sharding: core c = 4*b + g handles batch b (of 2) and head-group g (of 4).
Each group owns 8 query heads + 1 kv head (GQA 32q/4kv, head_dim 128) and the
matching column slice of w_qkv / row slice of w_o. The o_proj partial sums are
reduced on the host (all-reduce equivalent).

Numerics (v4+): the two big GEMMs (qkv proj, o_proj) run as fp8e4 DoubleRow
matmuls (0.5 cycles/row, 2 contraction k-tiles per instruction = 4x bf16
FLOP rate) with 3-term error compensation:
    A@B ~= Aq@Bq + Ar@Bq + Aq@Br
where Aq = e4m3(A), Ar = e4m3(A - Aq) (unscaled residual), same for B.
Inputs are pre-scaled into e4m3's sweet spot (X*4, W*128, ctx*4) with the
scales folded into existing multiplicative knobs (psum->sbuf copy scale, the
0.25-scaled ones matrix for softmax denominators, the o_proj output copy
scale). Attention (scores, ctx) stays bf16. Output partials ship bf16.

Schedule (v11): attention q-block j=0 and ALL RoPE are interleaved into
phase 1's instruction stream. j=0 only needs key tiles 0..3 (ready after the
first 4-token-tile quarter) and is ACT/DVE-latency-bound standalone; inside
p1 its elementwise work rides under the fp8 matmul stream and its small PE
bits fill p1's DMA-gated gaps. RoPE runs in dve_add form (1 PE matmul per
chunk) per-quarter as soon as each 512-token chunk of rk/rq[h] is
transposed. Phase 2 is attention j=1..3 with o_proj of q-block j-1
interleaved (hb-major, 12 fp8-DR matmuls per item), plus the j=3 o_proj
tail. PSUM: p1(2) + transpose(2) + scores(2) + ctx(2) banks during p1;
scores2(2) + o_proj(2) replace p1+transpose in phase 2.
"""
import sys

sys.path.insert(0, "/opt/trn_rl_repo")

import numpy as np

HIDDEN = 4096
N_Q_HEADS = 32
N_KV_HEADS = 4
HEAD_DIM = 128
ROPE_THETA = 500000.0
Q_SIZE = N_Q_HEADS * HEAD_DIM  # 4096
KV_SIZE = N_KV_HEADS * HEAD_DIM  # 512
B = 2
S = 2048
N_CORES = 8
N_GROUPS = 4
HEADS_PER_GROUP = N_Q_HEADS // N_GROUPS  # 8
GROUP_Q = HEADS_PER_GROUP * HEAD_DIM  # 1024
QKV_G = GROUP_Q + 2 * HEAD_DIM  # 1280 columns of qkv per group
SCALE = HEAD_DIM ** -0.5
NK = HIDDEN // 128  # 32 contraction k-tiles
NKT = S // 128  # 16 token/key tiles per sequence
NQB = S // 512  # 4 q-blocks
NHB = HIDDEN // 512  # 8 output-hidden blocks

# fp8 compensation scales: X*4, W*128 -> qkv psum = 512*qkv;
# ctx*4 (via 0.25-scaled ones), wo*128 -> o_proj psum = 512*out.
X_PRE = 4.0
W_PRE = 128.0
QKV_SCL = 1.0 / (X_PRE * W_PRE)  # 2^-9 on qkv psum->sbuf copies
OST_SCL = 1.0 / (4.0 * W_PRE)  # 2^-9 on o_proj psum->out copies

_COMPILED = None
LAST_EXEC_NS = None
DEBUG_NO_INTERLEAVE = False
DEBUG_DUMP_CTX = False


def _build(phases=(1, 2, 3)):
    import concourse.mybir as mybir
    import concourse.tile as tile
    from concourse import bacc

    F32 = mybir.dt.float32
    F32R = mybir.dt.float32r
    BF16 = mybir.dt.bfloat16
    FP8 = mybir.dt.float8e4
    DR = mybir.MatmulPerfMode.DoubleRow

    nc = bacc.Bacc("TRN2", target_bir_lowering=False, debug=False, num_devices=N_CORES)

    # xq/xr: [tt, 128, NK, 128]; x*[tt, h, ko, t] = e4m3 hi/lo of
    # X_PRE * X[tt*128+t, ko*128+h] (contiguous 4KB per partition row)
    xq = nc.dram_tensor("xq", [NKT, 128, NK, 128], FP8, kind="ExternalInput").ap()
    xr = nc.dram_tensor("xr", [NKT, 128, NK, 128], FP8, kind="ExternalInput").ap()
    # wq/wr split per feature block (contiguous -> full-rate DMA descriptors):
    # w[s][fb]: [128, NK, fw]; hi/lo e4m3 of W_PRE*W^T[ko*128+p, c0+f]
    w_dram = {}
    for s in ("q", "r"):
        for fb, (c0, fw) in enumerate(((0, 256), (256, 512), (768, 512))):
            w_dram[(s, fb)] = nc.dram_tensor(
                f"w{s}{fb}", [128, NK, fw], FP8, kind="ExternalInput"
            ).ap()
    # woq/wor: [128, hk, 4096]; hi/lo e4m3 of W_PRE*w_o[o, g*1024+hk*128+p]
    woq = nc.dram_tensor("woq", [128, HEADS_PER_GROUP, HIDDEN], FP8,
                         kind="ExternalInput").ap()
    wor = nc.dram_tensor("wor", [128, HEADS_PER_GROUP, HIDDEN], FP8,
                         kind="ExternalInput").ap()
    # rope tables (bf16): cos2[2i]=cos2[2i+1]=cos; sin2[2i]=+sin, sin2[2i+1]=-sin
    cos2 = nc.dram_tensor("cos2", [HEAD_DIM, S], BF16, kind="ExternalInput").ap()
    sin2 = nc.dram_tensor("sin2", [HEAD_DIM, S], BF16, kind="ExternalInput").ap()
    swp = nc.dram_tensor("swp", [128, 128], BF16, kind="ExternalInput").ap()
    # ones scaled by 0.25 so rcp = 4/sum(p) and ctxb = 4*ctx (e4m3 sweet spot)
    ones = nc.dram_tensor("ones", [128, 128], F32R, kind="ExternalInput").ap()
    ident = nc.dram_tensor("ident", [128, 128], BF16, kind="ExternalInput").ap()
    # triangular edge mask: maskt[p, c] = 1 if c >= p else 0
    maskt = nc.dram_tensor("maskt", [128, 128], BF16, kind="ExternalInput").ap()
    out_part = nc.dram_tensor(
        "out_part", [NKT, NHB, 128, 512], BF16, kind="ExternalOutput"
    ).ap()
    ctx_dbg = None
    if DEBUG_DUMP_CTX:
        ctx_dbg = nc.dram_tensor(
            "ctx_dbg", [NQB, HEADS_PER_GROUP, 128, 512], BF16,
            kind="ExternalOutput"
        ).ap()

    FB = ((0, 256), (256, 512), (768, 512))  # (col0, width): kv | q0..3 | q4..7

    from contextlib import ExitStack

    with tile.TileContext(nc) as tc:
        with ExitStack() as _stk:
            # attention pools live for the whole kernel: q-block j=0 runs
            # interleaved inside phase 1
            cpool = _stk.enter_context(tc.tile_pool(name="consts", bufs=1))
            vpool = _stk.enter_context(tc.tile_pool(name="vsb", bufs=1))
            rqpool = _stk.enter_context(tc.tile_pool(name="rqsb", bufs=1))
            ropet = _stk.enter_context(tc.tile_pool(name="ropet", bufs=2))
            ptpool = _stk.enter_context(tc.tile_pool(name="pt", bufs=4))
            paccpool = _stk.enter_context(tc.tile_pool(name="pacc", bufs=2))
            rcppool = _stk.enter_context(tc.tile_pool(name="rcp", bufs=2))
            ctmppool = _stk.enter_context(tc.tile_pool(name="ctmp", bufs=2))
            ctxq8pool = _stk.enter_context(tc.tile_pool(name="ctxq8", bufs=2))
            ctxr8pool = _stk.enter_context(tc.tile_pool(name="ctxr8", bufs=2))
            ctxps = _stk.enter_context(tc.tile_pool(name="ctxps", bufs=2, space="PSUM"))
            swp_sb = cpool.tile([128, 128], BF16)
            ones_sb = cpool.tile([128, 128], F32R)
            id_sb = cpool.tile([128, 128], BF16)
            mask_sb = cpool.tile([128, 128], BF16)
            cos_sb = cpool.tile([128, S], BF16)
            sin_sb = cpool.tile([128, S], BF16)

            # persistent SBUF state
            v_sb = [vpool.tile([128, 128], BF16, name=f"v{i}") for i in range(NKT)]
            rq = [rqpool.tile([128, S], BF16, name=f"rq{i}") for i in range(HEADS_PER_GROUP)]
            rk = rqpool.tile([128, S], BF16)

            def rope_chunk(t, c, pspool, pstag):
                # t[:, cs] = swp@(t*sin') + t*cos; DVE does the final add
                cs = slice(c * 512, (c + 1) * 512)
                m_sin = ropet.tile([128, 512], BF16, tag="msin", name="msin")
                nc.vector.tensor_mul(m_sin, t[:, cs], sin_sb[:, cs])
                m_cos = ropet.tile([128, 512], BF16, tag="mcos", name="mcos")
                nc.vector.tensor_mul(m_cos, t[:, cs], cos_sb[:, cs])
                ps = pspool.tile([128, 512], F32, tag=pstag, name="ropeps")
                nc.tensor.matmul(ps, swp_sb, m_sin, start=True, stop=True)
                nc.vector.tensor_add(t[:, cs], ps, m_cos)

            def finalize(fin, r_pool, r_tag, pool_merge=False):
                pacc_a, pacc_b, ctx_ps, ctx_q, ctx_r, jh = fin
                acc = pacc_a
                if pacc_b is not None:
                    eng = nc.gpsimd if pool_merge else nc.vector
                    eng.tensor_add(
                        pacc_a, pacc_a.bitcast(F32), pacc_b.bitcast(F32)
                    )
                r_ps = r_pool.tile([128, 512], F32, tag=r_tag, name="rpst")
                nc.tensor.matmul(r_ps, ones_sb, acc, start=True, stop=True)
                rcp = rcppool.tile([128, 512], F32, tag="rcp", name="rcpt")
                nc.vector.reciprocal(rcp, r_ps)
                # ctxb = 4*ctx (ones are 0.25-scaled); fp8 split for o_proj
                ctmp = ctmppool.tile([128, 512], BF16, tag="ctmp", name="ctmpt")
                nc.vector.tensor_mul(ctmp, ctx_ps, rcp)
                nc.gpsimd.tensor_copy(ctx_q, ctmp)
                nc.vector.tensor_tensor(
                    ctx_r, ctmp, ctx_q, mybir.AluOpType.subtract
                )
                if ctx_dbg is not None:
                    nc.sync.dma_start(ctx_dbg[jh[0], jh[1]], ctmp)

            class AttnBlock:
                """Emits attention for one q-block j, one (head, key-tile)
                step at a time. step() returns False when exhausted."""

                def __init__(self, j, sc_pool, sc_tag, r_pool, r_tag, depth,
                             op=None, op_step=0.0):
                    self.j = j
                    self.nkt = 4 * (j + 1)
                    self.kt_order = (list(range(4 * j, 4 * j + 4))
                                     + list(range(4 * j)))
                    self.ctx_q8 = ctxq8pool.tile(
                        [128, HEADS_PER_GROUP, 512], FP8, tag="ctxq", name="ctxq8"
                    )
                    self.ctx_r8 = ctxr8pool.tile(
                        [128, HEADS_PER_GROUP, 512], FP8, tag="ctxr", name="ctxr8"
                    )
                    self.sc_pool = sc_pool
                    self.sc_tag = sc_tag
                    self.r_pool = r_pool
                    self.r_tag = r_tag
                    self.depth = depth
                    self.op = op
                    self.op_budget = 0.0
                    self.op_step = op_step
                    self.h = 0
                    self.i = 0
                    self.fin = None
                    self.pend = []
                    self._start_head()

                def _sc_emit(self, i):
                    kt = self.kt_order[i]
                    di = kt - 4 * self.j
                    col0 = di * 128 if di >= 0 else 0
                    sc_ps = self.sc_pool.tile(
                        [128, 512], F32, tag=self.sc_tag, name="scpst"
                    )
                    nc.tensor.matmul(
                        sc_ps[:, col0:],
                        rk[:, kt * 128:(kt + 1) * 128],
                        rq[self.h][:, self.j * 512 + col0:(self.j + 1) * 512],
                        start=True,
                        stop=True,
                    )
                    return sc_ps, kt, col0

                def _start_head(self):
                    self.ctx_ps = ctxps.tile([128, 512], F32, tag="ctxp", name="ctxpt")
                    self.pacc_a = paccpool.tile(
                        [128, 512], F32R, tag="pacca", name="pacca"
                    )
                    self.pacc_b = None
                    self.b_init = False
                    self.flip = False
                    self.pend = [self._sc_emit(0)]
                    while len(self.pend) < min(self.depth, self.nkt):
                        self.pend.append(self._sc_emit(len(self.pend)))
                    if self.fin is not None:
                        finalize(self.fin, self.r_pool, self.r_tag,
                                 pool_merge=(self.j == 0))
                        self.fin = None

                def step(self):
                    if self.h >= HEADS_PER_GROUP:
                        return False
                    i, j, nkt = self.i, self.j, self.nkt
                    sc_ps, kt, col0 = self.pend.pop(0)
                    di = kt - 4 * j
                    pt = ptpool.tile([128, 512], BF16, tag="pt", name="ptt")
                    nc.scalar.activation(
                        pt[:, col0:], sc_ps[:, col0:],
                        mybir.ActivationFunctionType.Exp,
                        scale=SCALE,
                    )
                    if di >= 0:  # diagonal: mask triangular edge
                        nc.vector.tensor_mul(
                            pt[:, col0:col0 + 128],
                            pt[:, col0:col0 + 128],
                            mask_sb,
                        )
                    # dual-chain denominator accumulation
                    if i == 0:
                        nc.gpsimd.tensor_copy(self.pacc_a, pt)
                    elif col0 == 0 and not self.b_init:
                        self.pacc_b = paccpool.tile(
                            [128, 512], F32R, tag="paccb", name="paccb"
                        )
                        nc.vector.tensor_copy(self.pacc_b, pt)
                        self.b_init = True
                    elif (not self.b_init) or self.flip:
                        nc.gpsimd.tensor_add(
                            self.pacc_a[:, col0:],
                            self.pacc_a[:, col0:].bitcast(F32),
                            pt[:, col0:],
                        )
                        self.flip = False
                    else:
                        nc.vector.tensor_add(
                            self.pacc_b[:, col0:],
                            self.pacc_b[:, col0:].bitcast(F32),
                            pt[:, col0:],
                        )
                        self.flip = True
                    if self.op is not None:
                        self.op_budget += self.op_step
                        n = int(self.op_budget)
                        self.op_budget -= n
                        self.op.emit(n)
                    nc.tensor.matmul(
                        self.ctx_ps[:, col0:],
                        v_sb[kt],
                        pt[:, col0:],
                        start=(i == 0),
                        stop=(i == nkt - 1),
                        skip_group_check=True,
                    )
                    if i + self.depth < nkt:
                        self.pend.append(self._sc_emit(i + self.depth))
                    self.i += 1
                    if self.i == nkt:
                        self.fin = (self.pacc_a, self.pacc_b, self.ctx_ps,
                                    self.ctx_q8[:, self.h, :],
                                    self.ctx_r8[:, self.h, :],
                                    (self.j, self.h))
                        self.h += 1
                        self.i = 0
                        if self.h < HEADS_PER_GROUP:
                            self._start_head()
                    return True

                def finish(self):
                    while self.step():
                        pass

                def flush_fin(self):
                    if self.fin is not None:
                        finalize(self.fin, self.r_pool, self.r_tag,
                                 pool_merge=(self.j == 0))
                        self.fin = None

            # ---------------- phase 1: qkv + j0 attention + all rope --------
            j0 = None
            if 2 in phases:
                pass  # j0 created mid-p1 once quarter 0's data is ready
            with ExitStack() as _stk1:
                wpool = _stk1.enter_context(tc.tile_pool(name="w", bufs=1))
                xqpool = _stk1.enter_context(tc.tile_pool(name="xq", bufs=4))
                xrpool = _stk1.enter_context(tc.tile_pool(name="xr", bufs=4))
                qspool = _stk1.enter_context(tc.tile_pool(name="qs", bufs=3))
                ktpool = _stk1.enter_context(tc.tile_pool(name="kt8", bufs=2))
                p1ps = _stk1.enter_context(tc.tile_pool(name="p1ps", bufs=3, space="PSUM"))
                tpps = _stk1.enter_context(tc.tile_pool(name="tp", bufs=2, space="PSUM"))
                scj0 = _stk1.enter_context(tc.tile_pool(name="scj0", bufs=1, space="PSUM"))

                def mk_j0():
                    # depth-1 score pipeline on a single bank: sc(i+1)'s
                    # matmul serializes behind exp(sc(i)), hidden by pacing
                    return AttnBlock(0, sc_pool=scj0, sc_tag="scj0",
                                     r_pool=p1ps, r_tag="p1", depth=1)
                w_tiles = {}
                x_tiles = {}

                def emit_x(tt):
                    # X rides the second HWDGE queue (ACT) so it fair-shares
                    # the DMA engines with the W stream on SP
                    xq_t = xqpool.tile([128, NK, 128], FP8, tag="xq", name="xqt")
                    nc.scalar.dma_start(xq_t, xq[tt])
                    xr_t = xrpool.tile([128, NK, 128], FP8, tag="xr", name="xrt")
                    nc.scalar.dma_start(xr_t, xr[tt])
                    x_tiles[tt] = (xq_t, xr_t)

                def p1_preamble():
                    for s in ("q", "r"):
                        for fb, (c0, fw) in enumerate(FB):
                            for kc in range(4):
                                wt = wpool.tile([128, 8, fw], FP8, name=f"w{s}{fb}_{kc}")
                                w_tiles[(s, fb, kc)] = wt

                    def wdma(s, fb, kc):
                        nc.sync.dma_start(
                            w_tiles[(s, fb, kc)],
                            w_dram[(s, fb)][:, kc * 8:(kc + 1) * 8, :],
                        )

                    # first W chunk in two halves so the PE starts sooner
                    nc.sync.dma_start(
                        w_tiles[("q", 0, 0)][:, :4, :], w_dram[("q", 0)][:, 0:4, :]
                    )
                    # x0 in two halves so the first kv matmuls start sooner
                    xq_t0 = xqpool.tile([128, NK, 128], FP8, tag="xq", name="xqt0")
                    nc.scalar.dma_start(xq_t0[:, :8], xq[0][:, :8])
                    nc.sync.dma_start(
                        w_tiles[("q", 0, 0)][:, 4:, :], w_dram[("q", 0)][:, 4:8, :]
                    )
                    nc.scalar.dma_start(xq_t0[:, 8:16], xq[0][:, 8:16])
                    wdma("q", 0, 1)
                    nc.scalar.dma_start(xq_t0[:, 16:], xq[0][:, 16:])
                    wdma("q", 0, 2), wdma("q", 0, 3)
                    xr_t0 = xrpool.tile([128, NK, 128], FP8, tag="xr", name="xrt0")
                    nc.scalar.dma_start(xr_t0, xr[0])
                    x_tiles[0] = (xq_t0, xr_t0)
                    for kc in range(4):
                        wdma("r", 0, kc)
                    nc.sync.dma_start(id_sb, ident)
                    emit_x(1)
                    emit_x(2)
                    emit_x(3)
                    for kc in range(4):
                        wdma("q", 1, kc)
                    for kc in range(4):
                        wdma("r", 1, kc)
                    for kc in range(4):
                        wdma("q", 2, kc)
                    for kc in range(4):
                        wdma("r", 2, kc)
                    # consts needed by rope/attention interleave from quarter 1
                    nc.sync.dma_start(swp_sb, swp)
                    nc.sync.dma_start(ones_sb, ones)
                    nc.sync.dma_start(mask_sb, maskt)
                    nc.sync.dma_start(cos_sb, cos2)
                    nc.sync.dma_start(sin_sb, sin2)

                def transpose_to(src):
                    tps = tpps.tile([128, 128], BF16, tag="tp", name="tps")
                    nc.tensor.transpose(tps, src, id_sb)
                    return tps

                pend_q = []  # deferred q transposes: (qs_tile, hh_base, tt)

                def p1_block(tt, fb):
                    c0, fw = FB[fb]
                    xq_t, xr_t = x_tiles[tt]
                    ps = p1ps.tile([128, 512], F32, tag="p1", name="p1t")
                    n_mm = 3 * (NK // 2)
                    mi = 0
                    # term order AqBq, ArBq, AqBr: xr arrives before wr via DMA
                    for a_t, w_s in ((xq_t, "q"), (xr_t, "q"), (xq_t, "r")):
                        for i in range(NK // 2):
                            kc, m = divmod(i, 4)
                            nc.tensor.matmul(
                                ps[:, :fw],
                                a_t[:, 2 * i:2 * i + 2, :],
                                w_tiles[(w_s, fb, kc)][:, 2 * m:2 * m + 2, :],
                                start=(mi == 0),
                                stop=(mi == n_mm - 1),
                                perf_mode=DR,
                            )
                            mi += 1
                    if fb == 0:
                        kt8 = ktpool.tile([128, 128], BF16, tag="kt8", name="kt8t")
                        nc.vector.tensor_scalar_mul(kt8, ps[:, :128], QKV_SCL)
                        nc.vector.tensor_scalar_mul(v_sb[tt], ps[:, 128:256], QKV_SCL)
                        tps = transpose_to(kt8)
                        nc.scalar.copy(rk[:, tt * 128:(tt + 1) * 128], tps)
                    else:
                        qs = qspool.tile([128, 512], BF16, tag="qs", name="qst")
                        nc.vector.tensor_scalar_mul(qs, ps[:, :fw], QKV_SCL)
                        pend_q.append((qs, (fb - 1) * 4, tt))
                    # drain one pending q-transpose batch per block
                    if len(pend_q) > 1:
                        qs_t, hh0, qtt = pend_q.pop(0)
                        for hh in range(4):
                            h = hh0 + hh
                            tps = transpose_to(qs_t[:, hh * 128:(hh + 1) * 128])
                            nc.scalar.copy(
                                rq[h][:, qtt * 128:(qtt + 1) * 128], tps
                            )

                def p1_run():
                    # rope chunk c of a tensor needs the transposes of token
                    # tiles 4c..4c+3: heads 0-3 drain by end of quarter c,
                    # heads 4-7 two blocks into quarter c+1
                    nonlocal j0
                    rope_ok = 2 in phases
                    groups = [(0, 4), (4, 8), (8, 12), (12, 16)]
                    rope_lo = 0  # chunks roped for rk/rq0-3
                    rope_hi = 0  # chunks roped for rq4-7
                    for gi, (g0, g1) in enumerate(groups):
                        nblk = 0
                        for fb in range(3):
                            for tt in range(g0, g1):
                                p1_block(tt, fb)
                                nblk += 1
                                # prefetch at fb2: with bufs=4 the new tile
                                # reuses x(tt)'s buffer, whose last reader
                                # (this very block) is now emitted -- the pool
                                # WAR tracking only sees already-emitted reads
                                if fb == 2 and tt + 4 < NKT:
                                    emit_x(tt + 4)
                                if gi > 0 and nblk == 2 and rope_ok:
                                    # force-drain pending transpose batches:
                                    # the len>1 guard leaves the last (h4-7,
                                    # tt_{4c+3}) batch pending, and roping a
                                    # chunk before its transposes are emitted
                                    # bakes un-roped q into those columns
                                    while pend_q:
                                        qs_t, hh0, qtt = pend_q.pop(0)
                                        for hh in range(4):
                                            h = hh0 + hh
                                            tps = transpose_to(
                                                qs_t[:, hh * 128:(hh + 1) * 128])
                                            nc.scalar.copy(
                                                rq[h][:, qtt * 128:(qtt + 1) * 128],
                                                tps)
                                    while rope_hi < rope_lo:
                                        for h in range(4, 8):
                                            rope_chunk(rq[h], rope_hi, p1ps, "p1")
                                        rope_hi += 1
                                    if j0 is None:
                                        j0 = mk_j0()
                                # paced j0 attention: its ACT/DVE chains ride
                                # under the fp8 matmul stream
                                if j0 is not None and nblk >= 3 and not DEBUG_NO_INTERLEAVE:
                                    j0.step()
                        if rope_ok:
                            rope_chunk(rk, gi, p1ps, "p1")
                            for h in range(4):
                                rope_chunk(rq[h], gi, p1ps, "p1")
                            rope_lo = gi + 1
                    return rope_hi, rope_lo

                rope_hi = rope_lo = 0
                if 1 in phases:
                    p1_preamble()
                    rope_hi, rope_lo = p1_run()
                for qs_t, hh0, qtt in pend_q:
                    for hh in range(4):
                        h = hh0 + hh
                        tps = transpose_to(qs_t[:, hh * 128:(hh + 1) * 128])
                        nc.scalar.copy(rq[h][:, qtt * 128:(qtt + 1) * 128], tps)
                if 1 in phases and 2 in phases:
                    # rope leftovers: heads 4-7 chunk 3
                    while rope_hi < rope_lo:
                        for h in range(4, 8):
                            rope_chunk(rq[h], rope_hi, p1ps, "p1")
                        rope_hi += 1
                    if j0 is None:
                        j0 = mk_j0()
                    j0.finish()
                    j0.flush_fin()

            # ---------------- phase 2/3: attention j>=1 + o_proj ------------
            with ExitStack() as _stk2:
                wopool = _stk2.enter_context(tc.tile_pool(name="wo", bufs=1))
                ostpool = _stk2.enter_context(tc.tile_pool(name="ost", bufs=4))
                scps = _stk2.enter_context(tc.tile_pool(name="scps", bufs=4, space="PSUM"))
                opps = _stk2.enter_context(tc.tile_pool(name="opps", bufs=2, space="PSUM"))
                woq_sb = wopool.tile([128, HEADS_PER_GROUP, HIDDEN], FP8)
                wor_sb = wopool.tile([128, HEADS_PER_GROUP, HIDDEN], FP8)
                if 3 in phases:
                    # hb-sliced in o_proj emission order
                    for hb in range(NHB):
                        nc.sync.dma_start(
                            woq_sb[:, :, hb * 512:(hb + 1) * 512],
                            woq[:, :, hb * 512:(hb + 1) * 512],
                        )
                        nc.sync.dma_start(
                            wor_sb[:, :, hb * 512:(hb + 1) * 512],
                            wor[:, :, hb * 512:(hb + 1) * 512],
                        )

                class OpEmitter:
                    def __init__(self, j, ctx_q8, ctx_r8):
                        self.items = [
                            (tl, hb)
                            for hb in range(NHB)
                            for tl in range(4)
                        ] if (3 in phases) else []
                        self.j = j
                        self.cq = ctx_q8
                        self.cr = ctx_r8
                        self.pos = 0

                    def emit(self, n):
                        # n counts DR-matmul triples (one head-pair, 3 terms)
                        for _ in range(n):
                            if self.pos >= 4 * len(self.items):
                                return
                            item, hp = divmod(self.pos, 4)
                            tl, hb = self.items[item]
                            ts = slice(tl * 128, (tl + 1) * 128)
                            hs = slice(2 * hp, 2 * hp + 2)
                            os_ = slice(hb * 512, (hb + 1) * 512)
                            if hp == 0:
                                self.ps = opps.tile([128, 512], F32, tag="op", name="opps")
                            for a_t, w_t in (
                                (self.cq, woq_sb),
                                (self.cq, wor_sb),
                                (self.cr, woq_sb),
                            ):
                                nc.tensor.matmul(
                                    self.ps,
                                    a_t[:, hs, ts],
                                    w_t[:, hs, os_],
                                    start=(hp == 0 and a_t is self.cq and w_t is woq_sb),
                                    stop=(hp == 3 and a_t is self.cr),
                                    perf_mode=DR,
                                )
                            if hp == 3:
                                ost = ostpool.tile([128, 512], BF16, tag="ost", name="ost")
                                if item % 2 == 0:
                                    nc.vector.tensor_scalar_mul(ost, self.ps, OST_SCL)
                                else:
                                    nc.scalar.activation(
                                        ost, self.ps,
                                        mybir.ActivationFunctionType.Copy,
                                        scale=OST_SCL,
                                    )
                                nc.sync.dma_start(
                                    out_part[self.j * 4 + tl, hb], ost
                                )
                            self.pos += 1

                    def flush(self):
                        self.emit(4 * len(self.items) - self.pos)

                prev_op = None
                if 2 in phases and 3 in phases and j0 is not None:
                    prev_op = OpEmitter(0, j0.ctx_q8, j0.ctx_r8)
                for j in range(1, NQB) if 2 in phases else []:
                    nkt_j = 4 * (j + 1)
                    # 128 DR-triples per j, paced over 8*nkt_j attention steps;
                    # j=1 starts with a small deficit so the first wo slices
                    # can land after the w pool frees
                    op_step = (16.0 / nkt_j) if prev_op is not None else 0.0
                    if DEBUG_NO_INTERLEAVE and prev_op is not None:
                        prev_op.flush()
                    blk = AttnBlock(j, sc_pool=scps, sc_tag="sc",
                                    r_pool=scps, r_tag="sc", depth=3,
                                    op=None if DEBUG_NO_INTERLEAVE else prev_op,
                                    op_step=op_step)
                    if prev_op is not None and j == 1:
                        blk.op_budget = -6.0
                    blk.finish()
                    blk.flush_fin()
                    if prev_op is not None:
                        prev_op.flush()
                    prev_op = OpEmitter(j, blk.ctx_q8, blk.ctx_r8)
                if 2 in phases and prev_op is not None:
                    prev_op.flush()

    nc.compile()
    return nc


def _host_inputs(positions, hidden_states, w_qkv, w_o):
    """Shard + fp8-split + lay out inputs for the 8 cores (c = 4*b + g)."""
    import ml_dtypes

    bf16 = ml_dtypes.bfloat16
    fp8 = ml_dtypes.float8_e4m3
    positions = np.asarray(positions)
    hidden_states = np.asarray(hidden_states, dtype=np.float32)
    w_qkv = np.asarray(w_qkv, dtype=np.float32)
    w_o = np.asarray(w_o, dtype=np.float32)

    def split8(a):
        hi = a.astype(fp8)
        lo = (a - hi.astype(np.float32)).astype(fp8)
        return hi, lo

    inv_freq = 1.0 / (ROPE_THETA ** (np.arange(0, HEAD_DIM, 2, dtype=np.float64) / HEAD_DIM))
    ang = positions.astype(np.float64)[None, :] * inv_freq[:, None]  # [half, S]
    c = np.cos(ang).astype(np.float32)
    s = np.sin(ang).astype(np.float32)
    cos2 = np.empty((HEAD_DIM, S), dtype=np.float32)
    sin2 = np.empty((HEAD_DIM, S), dtype=np.float32)
    cos2[0::2] = c
    cos2[1::2] = c
    sin2[0::2] = s
    sin2[1::2] = -s

    swp = np.zeros((128, 128), dtype=np.float32)
    idx = np.arange(0, 128, 2)
    swp[idx, idx + 1] = 1.0
    swp[idx + 1, idx] = 1.0
    ones = np.full((128, 128), 0.25, dtype=np.float32)
    ident = np.eye(128, dtype=np.float32)
    maskt = (np.arange(128)[None, :] >= np.arange(128)[:, None]).astype(np.float32)

    xqs, xrs = [], []
    for b in range(B):
        xt_t = np.ascontiguousarray(
            (X_PRE * hidden_states[b]).reshape(NKT, 128, NK, 128).transpose(0, 3, 2, 1)
        )  # [tt, h, ko, t] f32
        hi, lo = split8(xt_t)
        xqs.append(hi)
        xrs.append(lo)

    wqs, wrs, woqs, wors = [], [], [], []
    for g in range(N_GROUPS):
        cols = np.concatenate([
            np.arange(Q_SIZE + g * HEAD_DIM, Q_SIZE + (g + 1) * HEAD_DIM),  # k
            np.arange(Q_SIZE + KV_SIZE + g * HEAD_DIM, Q_SIZE + KV_SIZE + (g + 1) * HEAD_DIM),  # v
            np.arange(g * GROUP_Q, (g + 1) * GROUP_Q),  # q0..q7
        ])
        wq_g = W_PRE * w_qkv[cols, :]  # [1280, 4096]
        wqkvt_t = np.ascontiguousarray(
            wq_g.T.reshape(NK, 128, QKV_G).transpose(1, 0, 2)
        )
        hi, lo = split8(wqkvt_t)
        wqs.append(hi)
        wrs.append(lo)  # each [128, NK, 1280]; sliced per fb below
        wot_full = W_PRE * w_o[:, g * GROUP_Q:(g + 1) * GROUP_Q].T  # [1024, 4096]
        wot_t = np.ascontiguousarray(
            wot_full.reshape(HEADS_PER_GROUP, 128, HIDDEN).transpose(1, 0, 2)
        )
        hi, lo = split8(wot_t)
        woqs.append(hi)
        wors.append(lo)

    FBH = ((0, 256), (256, 512), (768, 512))
    in_maps = []
    for c_id in range(N_CORES):
        b, g = divmod(c_id, N_GROUPS)
        wmap = {}
        for s, arr in (("q", wqs[g]), ("r", wrs[g])):
            for fb, (c0, fw) in enumerate(FBH):
                wmap[f"w{s}{fb}"] = np.ascontiguousarray(arr[:, :, c0:c0 + fw])
        in_maps.append({
            "xq": xqs[b],
            "xr": xrs[b],
            **wmap,
            "woq": woqs[g],
            "wor": wors[g],
            "cos2": cos2.astype(bf16),
            "sin2": sin2.astype(bf16),
            "swp": swp.astype(bf16),
            "ones": ones,
            "ident": ident.astype(bf16),
            "maskt": maskt.astype(bf16),
        })
    return in_maps


def kernel(positions, hidden_states, w_qkv, w_o):
    global _COMPILED, LAST_EXEC_NS
    from concourse import bass_utils

    if _COMPILED is None:
        _COMPILED = _build()
    nc = _COMPILED

    in_maps = _host_inputs(positions, hidden_states, w_qkv, w_o)
    res = bass_utils.run_bass_kernel_spmd(
        nc, in_maps, core_ids=list(range(N_CORES))
    )
    LAST_EXEC_NS = res.exec_time_ns

    out = np.zeros((B, S, HIDDEN), dtype=np.float32)
    for c_id in range(N_CORES):
        b = c_id // N_GROUPS
        part = res.results[c_id]["out_part"]  # [NKT, NHB, 128, 512] bf16
        out[b] += part.astype(np.float32).transpose(0, 2, 1, 3).reshape(S, HIDDEN)
    return out
